# revision 27
# baseline (speedup 1.0000x reference)
"""Bahdanau additive attention on 8 Trainium2 NeuronCores.

Math (per batch element b):
    ep = enc @ W1 + b1                      # [S, U]
    dp = dec @ W2 + b2                      # [T, U]
    score[t,s,:] = tanh(ep[s,:] + dp[t,:]) + (1-mask[s])*NEG
    logits[t,s] = score[t,s,:] @ Wv + bv    # [T, S]
    weights = softmax_s(logits)             # [T, S]
    context = weights @ enc                 # [T, D]

Sharding: data-parallel over batch B=8, one batch element per core.

Per-core pipeline:
  stage A (PE/DVE, fp32): transpose enc/dec via PE; epT[u,s] / dpT[u,t]
    projections with W1/W2 natural-layout stationaries; epT stored bf16.
  stage B: X[u,(t,s)] = epT + dpT[:,t] via DVE tensor_scalar (bf16 tensor,
    fp32 per-partition scalar); tanh on ACT in [128, TG*256] instructions
    (bf16); reduction over u on PE: stationary = Wv u-chunk replicated to
    M=32 at col-group j=G%4, moving = tanh tile [128u, 512] (2 target steps),
    N=512 -> 32 replicated logit rows per col group, 4 two-step groups per
    PSUM bank, 4 u-chunk matmuls accumulating per group.  A K=1 "opener"
    matmul ones32 (x) qsw (start=True) applies the mask additive term
    (1-m[s])*NEG*sum(Wv) and opens each accumulation region.
  logits redistribution: one DVE copy per bank (PSUM->SBUF, all partitions),
    then SBUF->DRAM->SBUF DMA round-trip: the [1,512]-per-2-steps row layout
    concatenates to exactly t-major [T,S] order in DRAM.
  stage C ([t,s] layout, fp32): exp on ACT with fused accum_out row-sums;
    DVE reciprocal; weights = exp * recip (per-partition); context =
    (expT)^T @ enc on PE (expT via 2 PE transposes), scaled by recip.

Softmax skips max-subtraction: logits are O(1) sums of tanh*Wv and masked
entries arrive as -1e9*sum(Wv) pre-exp, matching the reference's
pre-Dense(1) mask add.  bv is omitted: softmax over s is invariant to
per-t constants (bv = 0 in the problem spec).
"""

import numpy as np

B, S, T, D, U = 8, 256, 128, 512, 512
NEG = -1.0e9
TG = 16  # target steps per X/Y tile
N_CORES = 8

_CACHED_NC = None


def _build_nc():
    from contextlib import ExitStack

    import concourse.bass as bass
    import concourse.tile as tile
    from concourse import bacc, mybir
    from concourse.masks import make_identity

    f32 = mybir.dt.float32
    bf16 = mybir.dt.bfloat16
    u8 = mybir.dt.uint8
    Act = mybir.ActivationFunctionType
    Alu = mybir.AluOpType

    nc = bacc.Bacc(
        "TRN2",
        target_bir_lowering=False,
        debug=False,
        enable_asserts=False,
        num_devices=N_CORES,
        num_swdge_queues=4,
    )

    enc_d = nc.dram_tensor("enc", [S, D], f32, kind="ExternalInput")
    dec_d = nc.dram_tensor("dec", [T, D], f32, kind="ExternalInput")
    mask_d = nc.dram_tensor("mask", [S], u8, kind="ExternalInput")
    w1_d = nc.dram_tensor("w1", [D, U], f32, kind="ExternalInput")
    b1_d = nc.dram_tensor("b1", [U], f32, kind="ExternalInput")
    w2_d = nc.dram_tensor("w2", [D, U], f32, kind="ExternalInput")
    b2_d = nc.dram_tensor("b2", [U], f32, kind="ExternalInput")
    wv_d = nc.dram_tensor("wv", [U], f32, kind="ExternalInput")
    lg_scratch = nc.dram_tensor("lg_scratch", [T * S], f32)
    ctx_d = nc.dram_tensor("ctx_out", [T, D], f32, kind="ExternalOutput")
    wout_d = nc.dram_tensor("w_out", [T, S], f32, kind="ExternalOutput")

    UC = U // 128  # 4 u chunks
    DC = D // 128  # 4 d chunks
    SB = S // 128  # 2 s blocks
    NTG = T // TG  # X/Y tile groups
    NG = T // 2    # 2-target-step groups
    NBANK = NG // 4  # logits psum banks (4 groups per bank)
    HALF = NBANK // 2  # redistribution granularity

    with tile.TileContext(nc) as tc, ExitStack() as ctx:
        singles = ctx.enter_context(tc.tile_pool(name="singles", bufs=1))

        # ---- constant / input loads -------------------------------------
        identity = singles.tile([128, 128], f32, tag="identity")
        make_identity(nc, identity)

        enc_sb = singles.tile([128, SB, D], f32, tag="enc_sb")  # [s, sb, d]
        enc_r = enc_d.ap().rearrange("(sb p) d -> p sb d", p=128)
        for sb in range(SB):
            nc.gpsimd.dma_start(enc_sb[:, sb, :], enc_r[:, sb, :])
        dec_sb = singles.tile([128, D], f32, tag="dec_sb")  # [t, d]
        nc.sync.dma_start(dec_sb[:], dec_d.ap())
        w1_sb = singles.tile([128, DC, U], f32, tag="w1_sb")  # [d, dc, u]
        w1_r = w1_d.ap().rearrange("(c p) u -> p c u", p=128)
        w2_sb = singles.tile([128, DC, U], f32, tag="w2_sb")
        w2_r = w2_d.ap().rearrange("(c p) u -> p c u", p=128)
        for dc in range(DC):
            nc.scalar.dma_start(w2_sb[:, dc, :], w2_r[:, dc, :])
        for dc in range(DC):
            nc.scalar.dma_start(w1_sb[:, dc, :], w1_r[:, dc, :])
        wv_col = singles.tile([128, UC], f32, tag="wv_col")  # [u, uc]
        nc.gpsimd.dma_start(wv_col[:], wv_d.ap().rearrange("(c p) -> p c", p=128))
        b1_col = singles.tile([128, UC], f32, tag="b1_col")
        nc.gpsimd.dma_start(b1_col[:], b1_d.ap().rearrange("(c p) -> p c", p=128))
        b2_col = singles.tile([128, UC], f32, tag="b2_col")
        nc.gpsimd.dma_start(b2_col[:], b2_d.ap().rearrange("(c p) -> p c", p=128))
        mask_row = singles.tile([1, S], u8, tag="mask_row")
        nc.gpsimd.dma_start(mask_row[:], mask_d.ap()[None, :])

        ones_col = singles.tile([128, 1], f32, tag="ones_col")
        nc.vector.memset(ones_col[:], 1.0)
        ones128_bf = singles.tile([1, 128], bf16, tag="ones128_bf")
        nc.vector.memset(ones128_bf[:], 1.0)

        # wv32_bf[:, uc, :]: Wv u-chunk replicated across 32 columns
        wv32_f = singles.tile([128, UC, 32], f32, tag="wv32_f")
        nc.vector.memset(wv32_f[:], 0.0)
        for uc in range(UC):
            nc.vector.tensor_scalar(
                out=wv32_f[:, uc, :], in0=wv32_f[:, uc, :],
                scalar1=wv_col[:, uc:uc + 1], scalar2=None, op0=Alu.add)
        wv32_bf = singles.tile([128, UC, 32], bf16, tag="wv32_bf")
        nc.vector.tensor_copy(wv32_bf[:], wv32_f[:])

        # ---- stage A ----------------------------------------------------
        with tc.tile_pool(name="psA", bufs=2, space="PSUM") as psA:
            # sum(Wv) -> [1,1]
            wv_fsum = singles.tile([128, 1], f32, tag="wv_fsum")
            nc.vector.tensor_reduce(
                out=wv_fsum[:], in_=wv_col[:], axis=mybir.AxisListType.X, op=Alu.add)
            sw_ps = psA.tile([1, 1], f32, tag="sw")
            nc.tensor.matmul(sw_ps[:], lhsT=wv_fsum[:], rhs=ones_col[:],
                             start=True, stop=True)
            sumwv = singles.tile([1, 1], f32, tag="sumwv")
            nc.vector.tensor_copy(sumwv[:], sw_ps[:])

            # qsw[1, 512]: (1-m[s])*NEG*sum(Wv), duplicated for 2 target steps
            mask_f = singles.tile([1, S], f32, tag="mask_f")
            nc.vector.tensor_copy(mask_f[:], mask_row[:])
            qneg_row = singles.tile([1, S], f32, tag="qneg_row")
            nc.vector.tensor_scalar(
                out=qneg_row[:], in0=mask_f[:], scalar1=-NEG, scalar2=NEG,
                op0=Alu.mult, op1=Alu.add)
            nc.vector.tensor_scalar_mul(qneg_row[:], in0=qneg_row[:], scalar1=sumwv[:])
            qsw_bf = singles.tile([1, 2, S], bf16, tag="qsw_bf")
            nc.vector.tensor_copy(qsw_bf[:, 0, :], qneg_row[:])
            nc.vector.tensor_copy(qsw_bf[:, 1, :], qneg_row[:])

            # decT + dpT first: dpT gates every stage-B add
            decT = singles.tile([128, DC, T], f32, tag="decT")
            for dc in range(DC):
                tp = psA.tile([128, 128], f32, tag="tp", name="tp")
                nc.tensor.transpose(tp[:], dec_sb[:, dc * 128:(dc + 1) * 128], identity[:])
                nc.vector.tensor_copy(decT[:, dc, :], tp[:])
            dpT = singles.tile([128, UC, T], f32, tag="dpT")
            for uc in range(UC):
                pd = psA.tile([128, S], f32, tag="proj", name="proj")
                for dc in range(DC):
                    nc.tensor.matmul(
                        pd[:, 0:T], lhsT=w2_sb[:, dc, uc * 128:(uc + 1) * 128],
                        rhs=decT[:, dc, :], start=(dc == 0), stop=(dc == DC - 1))
                nc.vector.tensor_scalar(
                    out=dpT[:, uc, :], in0=pd[:, 0:T], scalar1=b2_col[:, uc:uc + 1],
                    scalar2=None, op0=Alu.add)

            encT = singles.tile([128, DC, S], f32, tag="encT")
            for dc in range(DC):
                for sb in range(SB):
                    tp = psA.tile([128, 128], f32, tag="tp", name="tp")
                    nc.tensor.transpose(tp[:], enc_sb[:, sb, dc * 128:(dc + 1) * 128], identity[:])
                    nc.vector.tensor_copy(encT[:, dc, sb * 128:(sb + 1) * 128], tp[:])
            w1_bf = singles.tile([128, DC, U], bf16, tag="w1_bf")
            nc.scalar.copy(w1_bf[:], w1_sb[:])
            encT_bf = singles.tile([128, DC, S], bf16, tag="encT_bf")
            nc.scalar.copy(encT_bf[:], encT[:])
            epT = singles.tile([128, UC, S], bf16, tag="epT")
            for uc in range(UC):
                pe = psA.tile([128, S], f32, tag="proj", name="proj")
                for dc in range(DC):
                    nc.tensor.matmul(
                        pe[:], lhsT=w1_bf[:, dc, uc * 128:(uc + 1) * 128],
                        rhs=encT_bf[:, dc, :], start=(dc == 0), stop=(dc == DC - 1))
                nc.vector.tensor_scalar(
                    out=epT[:, uc, :], in0=pe[:], scalar1=b1_col[:, uc:uc + 1],
                    scalar2=None, op0=Alu.add)

        # ---- stage B ----------------------------------------------------
        xpool = ctx.enter_context(tc.tile_pool(name="xp", bufs=5))
        ypool = ctx.enter_context(tc.tile_pool(name="yp", bufs=2))
        lgps = ctx.enter_context(tc.tile_pool(name="lgps", bufs=4, space="PSUM"))
        rowbuf = singles.tile([128, NBANK * 512], f32, tag="rowbuf")

        psC = ctx.enter_context(tc.tile_pool(name="psC", bufs=1, space="PSUM"))
        logits_sb = singles.tile([128, S], f32, tag="logits_sb")
        exp_sb = singles.tile([128, S], f32, tag="exp_sb")
        sums = singles.tile([128, 1], f32, tag="sums")
        recip = singles.tile([128, 1], f32, tag="recip")
        w_sb = singles.tile([128, S], f32, tag="w_sb")
        expT = singles.tile([128, SB, 64], f32, tag="expT", name="expT")
        ctx_ps = psC.tile([128, D], f32, tag="ctxp")
        ctx_sb = singles.tile([128, D], f32, tag="ctx_sb")

        def stage_c_half(h):
            # t-range h*64..h*64+63 lives on partitions h*64..h*64+63
            p0 = h * 64
            sl = slice(p0, p0 + 64)
            nc.gpsimd.dma_start(
                logits_sb[sl, :],
                lg_scratch.ap()[p0 * S:(p0 + 64) * S]
                .rearrange("(t s) -> t s", s=S))
            nc.scalar.activation(exp_sb[sl, :], logits_sb[sl, :], Act.Exp,
                                 accum_out=sums[sl, :])
            nc.vector.reciprocal(recip[sl, :], sums[sl, :])
            nc.vector.tensor_scalar_mul(w_sb[sl, :], in0=exp_sb[sl, :],
                                        scalar1=recip[sl, :])
            nc.gpsimd.dma_start(wout_d.ap()[sl, :], w_sb[sl, :])
            for sb in range(SB):
                wps = psC.tile([128, 64], f32, tag="wps", name="wps")
                nc.tensor.transpose(wps[:], exp_sb[sl, sb * 128:(sb + 1) * 128],
                                    identity[sl, p0:p0 + 64],
                                    tile_position=(p0, 0))
                nc.vector.tensor_copy(expT[:, sb, :], wps[:])
                nc.tensor.matmul(
                    ctx_ps[sl, :], lhsT=expT[:, sb, :], rhs=enc_sb[:, sb, :],
                    start=(sb == 0), stop=(sb == SB - 1),
                    tile_position=(0, p0))
            nc.vector.tensor_scalar_mul(ctx_sb[sl, :], in0=ctx_ps[sl, :],
                                        scalar1=recip[sl, :])
            nc.gpsimd.dma_start(ctx_d.ap()[sl, :], ctx_sb[sl, :])

        gpt = TG // 2  # 2-step groups per tile group
        for tg in range(NTG):
            ys = []
            for uc in range(UC):
                y = ypool.tile([128, TG * S], bf16, tag=f"y{uc}", name=f"y{uc}")
                if (tg == 0 and uc >= 2) or (uc == UC - 1 and tg in (1, 2, 4, 6)):
                    # fused add+tanh on ACT (per-partition bias carries dpT)
                    for tl in range(TG):
                        t = tg * TG + tl
                        nc.scalar.activation(
                            y[:, tl * S:(tl + 1) * S], epT[:, uc, :], Act.Tanh,
                            bias=dpT[:, uc, t:t + 1])
                else:
                    x = xpool.tile([128, TG * S], bf16, tag="x", name="x")
                    for tl in range(TG):
                        t = tg * TG + tl
                        nc.vector.tensor_scalar(
                            out=x[:, tl * S:(tl + 1) * S], in0=epT[:, uc, :],
                            scalar1=dpT[:, uc, t:t + 1], scalar2=None, op0=Alu.add)
                    nc.scalar.activation(y[:], x[:], Act.Tanh)
                ys.append(y)
            # bank-wide openers (mask term, start=True over the whole bank)
            banks = []
            for bb in range(gpt // 4):
                rowbank = lgps.tile([128, 512], f32, tag="rowbank",
                                    name="rowbank")
                nc.tensor.matmul(
                    rowbank[:], lhsT=ones128_bf[:], rhs=qsw_bf[0:1, :, :],
                    start=True, stop=False)
                banks.append(rowbank)
            # u-chunk-major reduction: same stationary back-to-back
            for uc in range(UC):
                for gl in range(gpt):
                    G = tg * gpt + gl
                    j = G % 4
                    nc.tensor.matmul(
                        banks[gl // 4][32 * j:32 * (j + 1), :],
                        lhsT=wv32_bf[:, uc, :],
                        rhs=ys[uc][:, (2 * gl) * S:(2 * gl + 2) * S],
                        start=False, stop=(uc == UC - 1),
                        tile_position=(0, 32 * j))
            for bb in range(gpt // 4):
                b = tg * (gpt // 4) + bb
                nc.vector.tensor_copy(rowbuf[:, b * 512:(b + 1) * 512],
                                      banks[bb][:])
                if b % HALF == HALF - 1:
                    h0 = (b // HALF) * HALF
                    nc.gpsimd.dma_start(
                        lg_scratch.ap()[h0 * 2048:(h0 + HALF) * 2048]
                        .rearrange("(h j q) -> j h q", j=4, q=512),
                        rowbuf[0:97:32, h0 * 512:(h0 + HALF) * 512]
                        .rearrange("j (h q) -> j h q", q=512))
                    stage_c_half(b // HALF)



    nc.compile()
    return nc


def _get_nc():
    global _CACHED_NC
    if _CACHED_NC is None:
        _CACHED_NC = _build_nc()
    return _CACHED_NC


def kernel(encoder_output, decoder_output, mask, W1, b1, W2, b2, Wv, bv,
           _trace=False):
    from concourse.bass_utils import run_bass_kernel_spmd

    nc = _get_nc()
    W1 = np.ascontiguousarray(np.asarray(W1, dtype=np.float32))
    W2 = np.ascontiguousarray(np.asarray(W2, dtype=np.float32))
    b1 = np.ascontiguousarray(np.asarray(b1, dtype=np.float32))
    b2 = np.ascontiguousarray(np.asarray(b2, dtype=np.float32))
    wv = np.ascontiguousarray(np.asarray(Wv, dtype=np.float32).reshape(U))
    in_maps = []
    for b in range(B):
        in_maps.append({
            "enc": np.ascontiguousarray(np.asarray(encoder_output[b], dtype=np.float32)),
            "dec": np.ascontiguousarray(np.asarray(decoder_output[b], dtype=np.float32)),
            "mask": np.ascontiguousarray(np.asarray(mask[b]).astype(np.uint8)),
            "w1": W1, "b1": b1, "w2": W2, "b2": b2, "wv": wv,
        })
    res = run_bass_kernel_spmd(nc, in_maps, core_ids=list(range(N_CORES)),
                               trace=_trace)
    context = np.stack([res.results[b]["ctx_out"] for b in range(B)])
    weights = np.stack([res.results[b]["w_out"] for b in range(B)])[..., None]
    if _trace:
        kernel._last_result = res
    return context.astype(np.float32), weights.astype(np.float32)


# revision 28
# speedup vs baseline: 1.0117x; 1.0117x over previous
"""Bahdanau additive attention on 8 Trainium2 NeuronCores.

Math (per batch element b):
    ep = enc @ W1 + b1                      # [S, U]
    dp = dec @ W2 + b2                      # [T, U]
    score[t,s,:] = tanh(ep[s,:] + dp[t,:]) + (1-mask[s])*NEG
    logits[t,s] = score[t,s,:] @ Wv + bv    # [T, S]
    weights = softmax_s(logits)             # [T, S]
    context = weights @ enc                 # [T, D]

Sharding: data-parallel over batch B=8, one batch element per core.

Per-core pipeline:
  stage A (PE/DVE, fp32): transpose enc/dec via PE; epT[u,s] / dpT[u,t]
    projections with W1/W2 natural-layout stationaries; epT stored bf16.
  stage B: X[u,(t,s)] = epT + dpT[:,t] via DVE tensor_scalar (bf16 tensor,
    fp32 per-partition scalar); tanh on ACT in [128, TG*256] instructions
    (bf16); reduction over u on PE: stationary = Wv u-chunk replicated to
    M=32 at col-group j=G%4, moving = tanh tile [128u, 512] (2 target steps),
    N=512 -> 32 replicated logit rows per col group, 4 two-step groups per
    PSUM bank, 4 u-chunk matmuls accumulating per group.  A K=1 "opener"
    matmul ones32 (x) qsw (start=True) applies the mask additive term
    (1-m[s])*NEG*sum(Wv) and opens each accumulation region.
  logits redistribution: one DVE copy per bank (PSUM->SBUF, all partitions),
    then SBUF->DRAM->SBUF DMA round-trip: the [1,512]-per-2-steps row layout
    concatenates to exactly t-major [T,S] order in DRAM.
  stage C ([t,s] layout, fp32): exp on ACT with fused accum_out row-sums;
    DVE reciprocal; weights = exp * recip (per-partition); context =
    (expT)^T @ enc on PE (expT via 2 PE transposes), scaled by recip.

Softmax skips max-subtraction: logits are O(1) sums of tanh*Wv and masked
entries arrive as -1e9*sum(Wv) pre-exp, matching the reference's
pre-Dense(1) mask add.  bv is omitted: softmax over s is invariant to
per-t constants (bv = 0 in the problem spec).
"""

import numpy as np

B, S, T, D, U = 8, 256, 128, 512, 512
NEG = -1.0e9
TG = 16  # target steps per X/Y tile
N_CORES = 8

_CACHED_NC = None


def _build_nc():
    from contextlib import ExitStack

    import concourse.bass as bass
    import concourse.tile as tile
    from concourse import bacc, mybir
    from concourse.masks import make_identity

    f32 = mybir.dt.float32
    bf16 = mybir.dt.bfloat16
    u8 = mybir.dt.uint8
    Act = mybir.ActivationFunctionType
    Alu = mybir.AluOpType

    nc = bacc.Bacc(
        "TRN2",
        target_bir_lowering=False,
        debug=False,
        enable_asserts=False,
        num_devices=N_CORES,
        num_swdge_queues=4,
    )

    enc_d = nc.dram_tensor("enc", [S, D], f32, kind="ExternalInput")
    dec_d = nc.dram_tensor("dec", [T, D], f32, kind="ExternalInput")
    mask_d = nc.dram_tensor("mask", [S], u8, kind="ExternalInput")
    w1_d = nc.dram_tensor("w1", [D, U], f32, kind="ExternalInput")
    b1_d = nc.dram_tensor("b1", [U], f32, kind="ExternalInput")
    w2_d = nc.dram_tensor("w2", [D, U], f32, kind="ExternalInput")
    b2_d = nc.dram_tensor("b2", [U], f32, kind="ExternalInput")
    wv_d = nc.dram_tensor("wv", [U], f32, kind="ExternalInput")
    lg_scratch = nc.dram_tensor("lg_scratch", [T * S], f32)
    ctx_d = nc.dram_tensor("ctx_out", [T, D], f32, kind="ExternalOutput")
    wout_d = nc.dram_tensor("w_out", [T, S], f32, kind="ExternalOutput")

    UC = U // 128  # 4 u chunks
    DC = D // 128  # 4 d chunks
    SB = S // 128  # 2 s blocks
    NTG = T // TG  # X/Y tile groups
    NG = T // 2    # 2-target-step groups
    NBANK = NG // 4  # logits psum banks (4 groups per bank)
    HALF = NBANK // 2  # redistribution granularity

    with tile.TileContext(nc) as tc, ExitStack() as ctx:
        singles = ctx.enter_context(tc.tile_pool(name="singles", bufs=1))

        # ---- constant / input loads -------------------------------------
        identity = singles.tile([128, 128], f32, tag="identity")
        make_identity(nc, identity)

        enc_sb = singles.tile([128, SB, D], f32, tag="enc_sb")  # [s, sb, d]
        enc_r = enc_d.ap().rearrange("(sb p) d -> p sb d", p=128)
        for sb in range(SB):
            nc.gpsimd.dma_start(enc_sb[:, sb, :], enc_r[:, sb, :])
        dec_sb = singles.tile([128, D], f32, tag="dec_sb")  # [t, d]
        nc.sync.dma_start(dec_sb[:], dec_d.ap())
        w1_sb = singles.tile([128, DC, U], f32, tag="w1_sb")  # [d, dc, u]
        w1_r = w1_d.ap().rearrange("(c p) u -> p c u", p=128)
        w2_sb = singles.tile([128, DC, U], f32, tag="w2_sb")
        w2_r = w2_d.ap().rearrange("(c p) u -> p c u", p=128)
        for dc in range(DC):
            nc.scalar.dma_start(w2_sb[:, dc, :], w2_r[:, dc, :])
        for dc in range(DC):
            nc.scalar.dma_start(w1_sb[:, dc, :], w1_r[:, dc, :])
        wv_col = singles.tile([128, UC], f32, tag="wv_col")  # [u, uc]
        nc.gpsimd.dma_start(wv_col[:], wv_d.ap().rearrange("(c p) -> p c", p=128))
        b1_col = singles.tile([128, UC], f32, tag="b1_col")
        nc.gpsimd.dma_start(b1_col[:], b1_d.ap().rearrange("(c p) -> p c", p=128))
        b2_col = singles.tile([128, UC], f32, tag="b2_col")
        nc.gpsimd.dma_start(b2_col[:], b2_d.ap().rearrange("(c p) -> p c", p=128))
        mask_row = singles.tile([1, S], u8, tag="mask_row")
        nc.gpsimd.dma_start(mask_row[:], mask_d.ap()[None, :])

        ones_col = singles.tile([128, 1], f32, tag="ones_col")
        nc.vector.memset(ones_col[:], 1.0)
        ones128_bf = singles.tile([1, 128], bf16, tag="ones128_bf")
        nc.vector.memset(ones128_bf[:], 1.0)

        # wv32_bf[:, uc, :]: Wv u-chunk replicated across 32 columns
        wv32_f = singles.tile([128, UC, 32], f32, tag="wv32_f")
        nc.vector.memset(wv32_f[:], 0.0)
        for uc in range(UC):
            nc.vector.tensor_scalar(
                out=wv32_f[:, uc, :], in0=wv32_f[:, uc, :],
                scalar1=wv_col[:, uc:uc + 1], scalar2=None, op0=Alu.add)
        wv32_bf = singles.tile([128, UC, 32], bf16, tag="wv32_bf")
        nc.vector.tensor_copy(wv32_bf[:], wv32_f[:])

        # ---- stage A ----------------------------------------------------
        with tc.tile_pool(name="psA", bufs=2, space="PSUM") as psA:
            # sum(Wv) -> [1,1]
            wv_fsum = singles.tile([128, 1], f32, tag="wv_fsum")
            nc.vector.tensor_reduce(
                out=wv_fsum[:], in_=wv_col[:], axis=mybir.AxisListType.X, op=Alu.add)
            sw_ps = psA.tile([1, 1], f32, tag="sw")
            nc.tensor.matmul(sw_ps[:], lhsT=wv_fsum[:], rhs=ones_col[:],
                             start=True, stop=True)
            sumwv = singles.tile([1, 1], f32, tag="sumwv")
            nc.vector.tensor_copy(sumwv[:], sw_ps[:])

            # qsw[1, 512]: (1-m[s])*NEG*sum(Wv), duplicated for 2 target steps
            mask_f = singles.tile([1, S], f32, tag="mask_f")
            nc.vector.tensor_copy(mask_f[:], mask_row[:])
            qneg_row = singles.tile([1, S], f32, tag="qneg_row")
            nc.vector.tensor_scalar(
                out=qneg_row[:], in0=mask_f[:], scalar1=-NEG, scalar2=NEG,
                op0=Alu.mult, op1=Alu.add)
            nc.vector.tensor_scalar_mul(qneg_row[:], in0=qneg_row[:], scalar1=sumwv[:])
            qsw_bf = singles.tile([1, 2, S], bf16, tag="qsw_bf")
            nc.vector.tensor_copy(qsw_bf[:, 0, :], qneg_row[:])
            nc.vector.tensor_copy(qsw_bf[:, 1, :], qneg_row[:])

            # decT + dpT first: dpT gates every stage-B add
            decT = singles.tile([128, DC, T], f32, tag="decT")
            for dc in range(DC):
                tp = psA.tile([128, 128], f32, tag="tp", name="tp")
                nc.tensor.transpose(tp[:], dec_sb[:, dc * 128:(dc + 1) * 128], identity[:])
                nc.vector.tensor_copy(decT[:, dc, :], tp[:])
            dpT = singles.tile([128, UC, T], f32, tag="dpT")
            for uc in range(UC):
                pd = psA.tile([128, S], f32, tag="proj", name="proj")
                for dc in range(DC):
                    nc.tensor.matmul(
                        pd[:, 0:T], lhsT=w2_sb[:, dc, uc * 128:(uc + 1) * 128],
                        rhs=decT[:, dc, :], start=(dc == 0), stop=(dc == DC - 1))
                nc.vector.tensor_scalar(
                    out=dpT[:, uc, :], in0=pd[:, 0:T], scalar1=b2_col[:, uc:uc + 1],
                    scalar2=None, op0=Alu.add)

            encT = singles.tile([128, DC, S], f32, tag="encT")
            for dc in range(DC):
                for sb in range(SB):
                    tp = psA.tile([128, 128], f32, tag="tp", name="tp")
                    nc.tensor.transpose(tp[:], enc_sb[:, sb, dc * 128:(dc + 1) * 128], identity[:])
                    nc.vector.tensor_copy(encT[:, dc, sb * 128:(sb + 1) * 128], tp[:])
            w1_bf = singles.tile([128, DC, U], bf16, tag="w1_bf")
            nc.scalar.copy(w1_bf[:], w1_sb[:])
            encT_bf = singles.tile([128, DC, S], bf16, tag="encT_bf")
            nc.scalar.copy(encT_bf[:], encT[:])
            epT = singles.tile([128, UC, S], bf16, tag="epT")
            for uc in range(UC):
                pe = psA.tile([128, S], f32, tag="proj", name="proj")
                for dc in range(DC):
                    nc.tensor.matmul(
                        pe[:], lhsT=w1_bf[:, dc, uc * 128:(uc + 1) * 128],
                        rhs=encT_bf[:, dc, :], start=(dc == 0), stop=(dc == DC - 1))
                nc.vector.tensor_scalar(
                    out=epT[:, uc, :], in0=pe[:], scalar1=b1_col[:, uc:uc + 1],
                    scalar2=None, op0=Alu.add)

        # ---- stage B ----------------------------------------------------
        xpool = ctx.enter_context(tc.tile_pool(name="xp", bufs=5))
        ypool = ctx.enter_context(tc.tile_pool(name="yp", bufs=2))
        lgps = ctx.enter_context(tc.tile_pool(name="lgps", bufs=4, space="PSUM"))
        rowbuf = singles.tile([128, NBANK * 512], f32, tag="rowbuf")

        psC = ctx.enter_context(tc.tile_pool(name="psC", bufs=1, space="PSUM"))
        logits_sb = singles.tile([128, S], f32, tag="logits_sb")
        exp_sb = singles.tile([128, S], f32, tag="exp_sb")
        sums = singles.tile([128, 1], f32, tag="sums")
        recip = singles.tile([128, 1], f32, tag="recip")
        w_sb = singles.tile([128, S], f32, tag="w_sb")
        expT = singles.tile([128, SB, 64], f32, tag="expT", name="expT")
        ctx_ps = psC.tile([128, D], f32, tag="ctxp")
        ctx_sb = singles.tile([128, D], f32, tag="ctx_sb")

        def stage_c_half(h):
            # t-range h*64..h*64+63 lives on partitions h*64..h*64+63
            p0 = h * 64
            sl = slice(p0, p0 + 64)
            nc.gpsimd.dma_start(
                logits_sb[sl, :],
                lg_scratch.ap()[p0 * S:(p0 + 64) * S]
                .rearrange("(t s) -> t s", s=S))
            nc.scalar.activation(exp_sb[sl, :], logits_sb[sl, :], Act.Exp,
                                 accum_out=sums[sl, :])
            nc.vector.reciprocal(recip[sl, :], sums[sl, :])
            nc.vector.tensor_scalar_mul(w_sb[sl, :], in0=exp_sb[sl, :],
                                        scalar1=recip[sl, :])
            nc.gpsimd.dma_start(wout_d.ap()[sl, :], w_sb[sl, :])
            for sb in range(SB):
                wps = psC.tile([128, 64], f32, tag="wps", name="wps")
                nc.tensor.transpose(wps[:], exp_sb[sl, sb * 128:(sb + 1) * 128],
                                    identity[sl, p0:p0 + 64],
                                    tile_position=(p0, 0))
                nc.vector.tensor_copy(expT[:, sb, :], wps[:])
                nc.tensor.matmul(
                    ctx_ps[sl, :], lhsT=expT[:, sb, :], rhs=enc_sb[:, sb, :],
                    start=(sb == 0), stop=(sb == SB - 1),
                    tile_position=(0, p0))
            nc.vector.tensor_scalar_mul(ctx_sb[sl, :], in0=ctx_ps[sl, :],
                                        scalar1=recip[sl, :])
            nc.gpsimd.dma_start(ctx_d.ap()[sl, :], ctx_sb[sl, :])

        gpt = TG // 2  # 2-step groups per tile group
        for tg in range(NTG):
            ys = []
            for uc in range(UC):
                y = ypool.tile([128, TG * S], bf16, tag=f"y{uc}", name=f"y{uc}")
                if (tg == 0 and uc >= 2) or (uc == UC - 1 and tg in (2, 4, 6)):
                    # fused add+tanh on ACT (per-partition bias carries dpT)
                    for tl in range(TG):
                        t = tg * TG + tl
                        nc.scalar.activation(
                            y[:, tl * S:(tl + 1) * S], epT[:, uc, :], Act.Tanh,
                            bias=dpT[:, uc, t:t + 1])
                else:
                    x = xpool.tile([128, TG * S], bf16, tag="x", name="x")
                    for tl in range(TG):
                        t = tg * TG + tl
                        nc.vector.tensor_scalar(
                            out=x[:, tl * S:(tl + 1) * S], in0=epT[:, uc, :],
                            scalar1=dpT[:, uc, t:t + 1], scalar2=None, op0=Alu.add)
                    nc.scalar.activation(y[:], x[:], Act.Tanh)
                ys.append(y)
            # bank-wide openers (mask term, start=True over the whole bank)
            banks = []
            for bb in range(gpt // 4):
                rowbank = lgps.tile([128, 512], f32, tag="rowbank",
                                    name="rowbank")
                nc.tensor.matmul(
                    rowbank[:], lhsT=ones128_bf[:], rhs=qsw_bf[0:1, :, :],
                    start=True, stop=False)
                banks.append(rowbank)
            # u-chunk-major reduction: same stationary back-to-back
            for uc in range(UC):
                for gl in range(gpt):
                    G = tg * gpt + gl
                    j = G % 4
                    nc.tensor.matmul(
                        banks[gl // 4][32 * j:32 * (j + 1), :],
                        lhsT=wv32_bf[:, uc, :],
                        rhs=ys[uc][:, (2 * gl) * S:(2 * gl + 2) * S],
                        start=False, stop=(uc == UC - 1),
                        tile_position=(0, 32 * j))
            for bb in range(gpt // 4):
                b = tg * (gpt // 4) + bb
                nc.vector.tensor_copy(rowbuf[:, b * 512:(b + 1) * 512],
                                      banks[bb][:])
                if b % HALF == HALF - 1:
                    h0 = (b // HALF) * HALF
                    nc.gpsimd.dma_start(
                        lg_scratch.ap()[h0 * 2048:(h0 + HALF) * 2048]
                        .rearrange("(h j q) -> j h q", j=4, q=512),
                        rowbuf[0:97:32, h0 * 512:(h0 + HALF) * 512]
                        .rearrange("j (h q) -> j h q", q=512))
                    stage_c_half(b // HALF)



    nc.compile()
    return nc


def _get_nc():
    global _CACHED_NC
    if _CACHED_NC is None:
        _CACHED_NC = _build_nc()
    return _CACHED_NC


def kernel(encoder_output, decoder_output, mask, W1, b1, W2, b2, Wv, bv,
           _trace=False):
    from concourse.bass_utils import run_bass_kernel_spmd

    nc = _get_nc()
    W1 = np.ascontiguousarray(np.asarray(W1, dtype=np.float32))
    W2 = np.ascontiguousarray(np.asarray(W2, dtype=np.float32))
    b1 = np.ascontiguousarray(np.asarray(b1, dtype=np.float32))
    b2 = np.ascontiguousarray(np.asarray(b2, dtype=np.float32))
    wv = np.ascontiguousarray(np.asarray(Wv, dtype=np.float32).reshape(U))
    in_maps = []
    for b in range(B):
        in_maps.append({
            "enc": np.ascontiguousarray(np.asarray(encoder_output[b], dtype=np.float32)),
            "dec": np.ascontiguousarray(np.asarray(decoder_output[b], dtype=np.float32)),
            "mask": np.ascontiguousarray(np.asarray(mask[b]).astype(np.uint8)),
            "w1": W1, "b1": b1, "w2": W2, "b2": b2, "wv": wv,
        })
    res = run_bass_kernel_spmd(nc, in_maps, core_ids=list(range(N_CORES)),
                               trace=_trace)
    context = np.stack([res.results[b]["ctx_out"] for b in range(B)])
    weights = np.stack([res.results[b]["w_out"] for b in range(B)])[..., None]
    if _trace:
        kernel._last_result = res
    return context.astype(np.float32), weights.astype(np.float32)


# revision 30
# speedup vs baseline: 1.0203x; 1.0084x over previous
"""Bahdanau additive attention on 8 Trainium2 NeuronCores.

Math (per batch element b):
    ep = enc @ W1 + b1                      # [S, U]
    dp = dec @ W2 + b2                      # [T, U]
    score[t,s,:] = tanh(ep[s,:] + dp[t,:]) + (1-mask[s])*NEG
    logits[t,s] = score[t,s,:] @ Wv + bv    # [T, S]
    weights = softmax_s(logits)             # [T, S]
    context = weights @ enc                 # [T, D]

Sharding: data-parallel over batch B=8, one batch element per core.

Per-core pipeline:
  stage A (PE/DVE, fp32): transpose enc/dec via PE; epT[u,s] / dpT[u,t]
    projections with W1/W2 natural-layout stationaries; epT stored bf16.
  stage B: X[u,(t,s)] = epT + dpT[:,t] via DVE tensor_scalar (bf16 tensor,
    fp32 per-partition scalar); tanh on ACT in [128, TG*256] instructions
    (bf16); reduction over u on PE: stationary = Wv u-chunk replicated to
    M=32 at col-group j=G%4, moving = tanh tile [128u, 512] (2 target steps),
    N=512 -> 32 replicated logit rows per col group, 4 two-step groups per
    PSUM bank, 4 u-chunk matmuls accumulating per group.  A K=1 "opener"
    matmul ones32 (x) qsw (start=True) applies the mask additive term
    (1-m[s])*NEG*sum(Wv) and opens each accumulation region.
  logits redistribution: one DVE copy per bank (PSUM->SBUF, all partitions),
    then SBUF->DRAM->SBUF DMA round-trip: the [1,512]-per-2-steps row layout
    concatenates to exactly t-major [T,S] order in DRAM.
  stage C ([t,s] layout, fp32): exp on ACT with fused accum_out row-sums;
    DVE reciprocal; weights = exp * recip (per-partition); context =
    (expT)^T @ enc on PE (expT via 2 PE transposes), scaled by recip.

Softmax skips max-subtraction: logits are O(1) sums of tanh*Wv and masked
entries arrive as -1e9*sum(Wv) pre-exp, matching the reference's
pre-Dense(1) mask add.  bv is omitted: softmax over s is invariant to
per-t constants (bv = 0 in the problem spec).
"""

import numpy as np

B, S, T, D, U = 8, 256, 128, 512, 512
NEG = -1.0e9
TG = 16  # target steps per X/Y tile
N_CORES = 8

_CACHED_NC = None


def _build_nc():
    from contextlib import ExitStack

    import concourse.bass as bass
    import concourse.tile as tile
    from concourse import bacc, mybir
    from concourse.masks import make_identity

    f32 = mybir.dt.float32
    bf16 = mybir.dt.bfloat16
    u8 = mybir.dt.uint8
    Act = mybir.ActivationFunctionType
    Alu = mybir.AluOpType

    nc = bacc.Bacc(
        "TRN2",
        target_bir_lowering=False,
        debug=False,
        enable_asserts=False,
        num_devices=N_CORES,
        num_swdge_queues=4,
    )

    enc_d = nc.dram_tensor("enc", [S, D], f32, kind="ExternalInput")
    dec_d = nc.dram_tensor("dec", [T, D], f32, kind="ExternalInput")
    mask_d = nc.dram_tensor("mask", [S], u8, kind="ExternalInput")
    w1_d = nc.dram_tensor("w1", [D, U], f32, kind="ExternalInput")
    b1_d = nc.dram_tensor("b1", [U], f32, kind="ExternalInput")
    w2_d = nc.dram_tensor("w2", [D, U], f32, kind="ExternalInput")
    b2_d = nc.dram_tensor("b2", [U], f32, kind="ExternalInput")
    wv_d = nc.dram_tensor("wv", [U], f32, kind="ExternalInput")
    lg_scratch = nc.dram_tensor("lg_scratch", [T * S], f32)
    ctx_d = nc.dram_tensor("ctx_out", [T, D], f32, kind="ExternalOutput")
    wout_d = nc.dram_tensor("w_out", [T, S], f32, kind="ExternalOutput")

    UC = U // 128  # 4 u chunks
    DC = D // 128  # 4 d chunks
    SB = S // 128  # 2 s blocks
    NTG = T // TG  # X/Y tile groups
    NG = T // 2    # 2-target-step groups
    NBANK = NG // 4  # logits psum banks (4 groups per bank)
    HALF = NBANK // 2  # redistribution granularity

    with tile.TileContext(nc) as tc, ExitStack() as ctx:
        singles = ctx.enter_context(tc.tile_pool(name="singles", bufs=1))

        # ---- constant / input loads -------------------------------------
        identity = singles.tile([128, 128], f32, tag="identity")
        make_identity(nc, identity)

        enc_sb = singles.tile([128, SB, D], f32, tag="enc_sb")  # [s, sb, d]
        enc_r = enc_d.ap().rearrange("(sb p) d -> p sb d", p=128)
        for sb in range(SB):
            nc.gpsimd.dma_start(enc_sb[:, sb, :], enc_r[:, sb, :])
        dec_sb = singles.tile([128, D], f32, tag="dec_sb")  # [t, d]
        nc.sync.dma_start(dec_sb[:], dec_d.ap())
        w1_sb = singles.tile([128, DC, U], f32, tag="w1_sb")  # [d, dc, u]
        w1_r = w1_d.ap().rearrange("(c p) u -> p c u", p=128)
        w2_sb = singles.tile([128, DC, U], f32, tag="w2_sb")
        w2_r = w2_d.ap().rearrange("(c p) u -> p c u", p=128)
        for dc in range(DC):
            nc.scalar.dma_start(w2_sb[:, dc, :], w2_r[:, dc, :])
        for dc in range(DC):
            nc.scalar.dma_start(w1_sb[:, dc, :], w1_r[:, dc, :])
        wv_col = singles.tile([128, UC], f32, tag="wv_col")  # [u, uc]
        nc.gpsimd.dma_start(wv_col[:], wv_d.ap().rearrange("(c p) -> p c", p=128))
        b1_col = singles.tile([128, UC], f32, tag="b1_col")
        nc.gpsimd.dma_start(b1_col[:], b1_d.ap().rearrange("(c p) -> p c", p=128))
        b2_col = singles.tile([128, UC], f32, tag="b2_col")
        nc.gpsimd.dma_start(b2_col[:], b2_d.ap().rearrange("(c p) -> p c", p=128))
        mask_row = singles.tile([1, S], u8, tag="mask_row")
        nc.gpsimd.dma_start(mask_row[:], mask_d.ap()[None, :])

        ones_col = singles.tile([128, 1], f32, tag="ones_col")
        nc.vector.memset(ones_col[:], 1.0)
        ones128_bf = singles.tile([1, 128], bf16, tag="ones128_bf")
        nc.vector.memset(ones128_bf[:], 1.0)

        # wv32_bf[:, uc, :]: Wv u-chunk replicated across 32 columns
        wv32_f = singles.tile([128, UC, 32], f32, tag="wv32_f")
        nc.vector.memset(wv32_f[:], 0.0)
        for uc in range(UC):
            nc.vector.tensor_scalar(
                out=wv32_f[:, uc, :], in0=wv32_f[:, uc, :],
                scalar1=wv_col[:, uc:uc + 1], scalar2=None, op0=Alu.add)
        wv32_bf = singles.tile([128, UC, 32], bf16, tag="wv32_bf")
        nc.vector.tensor_copy(wv32_bf[:], wv32_f[:])

        # ---- stage A ----------------------------------------------------
        with tc.tile_pool(name="psA", bufs=2, space="PSUM") as psA:
            # sum(Wv) -> [1,1]
            wv_fsum = singles.tile([128, 1], f32, tag="wv_fsum")
            nc.vector.tensor_reduce(
                out=wv_fsum[:], in_=wv_col[:], axis=mybir.AxisListType.X, op=Alu.add)
            sw_ps = psA.tile([1, 1], f32, tag="sw")
            nc.tensor.matmul(sw_ps[:], lhsT=wv_fsum[:], rhs=ones_col[:],
                             start=True, stop=True)
            sumwv = singles.tile([1, 1], f32, tag="sumwv")
            nc.vector.tensor_copy(sumwv[:], sw_ps[:])

            # qsw[1, 512]: (1-m[s])*NEG*sum(Wv), duplicated for 2 target steps
            mask_f = singles.tile([1, S], f32, tag="mask_f")
            nc.vector.tensor_copy(mask_f[:], mask_row[:])
            qneg_row = singles.tile([1, S], f32, tag="qneg_row")
            nc.vector.tensor_scalar(
                out=qneg_row[:], in0=mask_f[:], scalar1=-NEG, scalar2=NEG,
                op0=Alu.mult, op1=Alu.add)
            nc.vector.tensor_scalar_mul(qneg_row[:], in0=qneg_row[:], scalar1=sumwv[:])
            qsw_bf = singles.tile([1, 2, S], bf16, tag="qsw_bf")
            nc.vector.tensor_copy(qsw_bf[:, 0, :], qneg_row[:])
            nc.vector.tensor_copy(qsw_bf[:, 1, :], qneg_row[:])

            # decT + dpT first: dpT gates every stage-B add
            decT = singles.tile([128, DC, T], f32, tag="decT")
            for dc in range(DC):
                tp = psA.tile([128, 128], f32, tag="tp", name="tp")
                nc.tensor.transpose(tp[:], dec_sb[:, dc * 128:(dc + 1) * 128], identity[:])
                nc.vector.tensor_copy(decT[:, dc, :], tp[:])
            dpT = singles.tile([128, UC, T], f32, tag="dpT")
            for uc in range(UC):
                pd = psA.tile([128, S], f32, tag="proj", name="proj")
                for dc in range(DC):
                    nc.tensor.matmul(
                        pd[:, 0:T], lhsT=w2_sb[:, dc, uc * 128:(uc + 1) * 128],
                        rhs=decT[:, dc, :], start=(dc == 0), stop=(dc == DC - 1))
                nc.vector.tensor_scalar(
                    out=dpT[:, uc, :], in0=pd[:, 0:T], scalar1=b2_col[:, uc:uc + 1],
                    scalar2=None, op0=Alu.add)

            encT = singles.tile([128, DC, S], f32, tag="encT")
            for dc in range(DC):
                for sb in range(SB):
                    tp = psA.tile([128, 128], f32, tag="tp", name="tp")
                    nc.tensor.transpose(tp[:], enc_sb[:, sb, dc * 128:(dc + 1) * 128], identity[:])
                    nc.vector.tensor_copy(encT[:, dc, sb * 128:(sb + 1) * 128], tp[:])
            w1_bf = singles.tile([128, DC, U], bf16, tag="w1_bf")
            nc.scalar.copy(w1_bf[:], w1_sb[:])
            encT_bf = singles.tile([128, DC, S], bf16, tag="encT_bf")
            nc.scalar.copy(encT_bf[:], encT[:])
            epT = singles.tile([128, UC, S], bf16, tag="epT")
            for uc in range(UC):
                pe = psA.tile([128, S], f32, tag="proj", name="proj")
                for dc in range(DC):
                    nc.tensor.matmul(
                        pe[:], lhsT=w1_bf[:, dc, uc * 128:(uc + 1) * 128],
                        rhs=encT_bf[:, dc, :], start=(dc == 0), stop=(dc == DC - 1))
                nc.vector.tensor_scalar(
                    out=epT[:, uc, :], in0=pe[:], scalar1=b1_col[:, uc:uc + 1],
                    scalar2=None, op0=Alu.add)

        # ---- stage B ----------------------------------------------------
        xpool = ctx.enter_context(tc.tile_pool(name="xp", bufs=5))
        ypool = ctx.enter_context(tc.tile_pool(name="yp", bufs=2))
        lgps = ctx.enter_context(tc.tile_pool(name="lgps", bufs=4, space="PSUM"))
        rowbuf = singles.tile([128, NBANK * 512], f32, tag="rowbuf")

        psC = ctx.enter_context(tc.tile_pool(name="psC", bufs=1, space="PSUM"))
        logits_sb = singles.tile([128, S], f32, tag="logits_sb")
        exp_sb = singles.tile([128, S], f32, tag="exp_sb")
        sums = singles.tile([128, 1], f32, tag="sums")
        recip = singles.tile([128, 1], f32, tag="recip")
        w_sb = singles.tile([128, S], f32, tag="w_sb")
        expT = singles.tile([128, SB, 64], f32, tag="expT", name="expT")
        ctx_ps = psC.tile([128, D], f32, tag="ctxp")
        ctx_sb = singles.tile([128, D], f32, tag="ctx_sb")

        def stage_c_half(h):
            # t-range h*64..h*64+63 lives on partitions h*64..h*64+63
            p0 = h * 64
            sl = slice(p0, p0 + 64)
            nc.gpsimd.dma_start(
                logits_sb[sl, :],
                lg_scratch.ap()[p0 * S:(p0 + 64) * S]
                .rearrange("(t s) -> t s", s=S))
            nc.scalar.activation(exp_sb[sl, :], logits_sb[sl, :], Act.Exp,
                                 accum_out=sums[sl, :])
            nc.vector.reciprocal(recip[sl, :], sums[sl, :])
            nc.vector.tensor_scalar_mul(w_sb[sl, :], in0=exp_sb[sl, :],
                                        scalar1=recip[sl, :])
            nc.gpsimd.dma_start(wout_d.ap()[sl, :], w_sb[sl, :])
            for sb in range(SB):
                wps = psC.tile([128, 64], f32, tag="wps", name="wps")
                nc.tensor.transpose(wps[:], exp_sb[sl, sb * 128:(sb + 1) * 128],
                                    identity[sl, p0:p0 + 64],
                                    tile_position=(p0, 0))
                nc.vector.tensor_copy(expT[:, sb, :], wps[:])
                nc.tensor.matmul(
                    ctx_ps[sl, :], lhsT=expT[:, sb, :], rhs=enc_sb[:, sb, :],
                    start=(sb == 0), stop=(sb == SB - 1),
                    tile_position=(0, p0))
            nc.vector.tensor_scalar_mul(ctx_sb[sl, :], in0=ctx_ps[sl, :],
                                        scalar1=recip[sl, :])
            nc.gpsimd.dma_start(ctx_d.ap()[sl, :], ctx_sb[sl, :])

        gpt = TG // 2  # 2-step groups per tile group
        for tg in range(NTG):
            ys = []
            for uc in range(UC):
                y = ypool.tile([128, TG * S], bf16, tag=f"y{uc}", name=f"y{uc}")
                if (tg == 0 and uc >= 2) or (uc == UC - 1 and tg in (2, 4, 6)):
                    # fused add+tanh on ACT (per-partition bias carries dpT)
                    for tl in range(TG):
                        t = tg * TG + tl
                        nc.scalar.activation(
                            y[:, tl * S:(tl + 1) * S], epT[:, uc, :], Act.Tanh,
                            bias=dpT[:, uc, t:t + 1])
                else:
                    x = xpool.tile([128, TG * S], bf16, tag="x", name="x")
                    for tl in range(TG):
                        t = tg * TG + tl
                        nc.vector.tensor_scalar(
                            out=x[:, tl * S:(tl + 1) * S], in0=epT[:, uc, :],
                            scalar1=dpT[:, uc, t:t + 1], scalar2=None, op0=Alu.add)
                    nc.scalar.activation(y[:], x[:], Act.Tanh)
                ys.append(y)
            # bank-wide openers (mask term, start=True over the whole bank)
            banks = []
            for bb in range(gpt // 4):
                rowbank = lgps.tile([128, 512], f32, tag="rowbank",
                                    name="rowbank")
                nc.tensor.matmul(
                    rowbank[:], lhsT=ones128_bf[:], rhs=qsw_bf[0:1, :, :],
                    start=True, stop=False)
                banks.append(rowbank)
            # u-chunk-major reduction: same stationary back-to-back
            for uc in range(UC):
                for gl in range(gpt):
                    G = tg * gpt + gl
                    j = G % 4
                    nc.tensor.matmul(
                        banks[gl // 4][32 * j:32 * (j + 1), :],
                        lhsT=wv32_bf[:, uc, :],
                        rhs=ys[uc][:, (2 * gl) * S:(2 * gl + 2) * S],
                        start=False, stop=(uc == UC - 1),
                        tile_position=(0, 32 * j))
            for bb in range(gpt // 4):
                b = tg * (gpt // 4) + bb
                nc.vector.tensor_copy(rowbuf[:, b * 512:(b + 1) * 512],
                                      banks[bb][:])
                if b % HALF == HALF - 1:
                    h0 = (b // HALF) * HALF
                    nc.gpsimd.dma_start(
                        lg_scratch.ap()[h0 * 2048:(h0 + HALF) * 2048]
                        .rearrange("(h j q) -> j h q", j=4, q=512),
                        rowbuf[0:97:32, h0 * 512:(h0 + HALF) * 512]
                        .rearrange("j (h q) -> j h q", q=512))
                    stage_c_half(b // HALF)



    nc.compile()
    return nc


def _get_nc():
    global _CACHED_NC
    if _CACHED_NC is None:
        _CACHED_NC = _build_nc()
    return _CACHED_NC


def kernel(encoder_output, decoder_output, mask, W1, b1, W2, b2, Wv, bv,
           _trace=False):
    from concourse.bass_utils import run_bass_kernel_spmd

    nc = _get_nc()
    W1 = np.ascontiguousarray(np.asarray(W1, dtype=np.float32))
    W2 = np.ascontiguousarray(np.asarray(W2, dtype=np.float32))
    b1 = np.ascontiguousarray(np.asarray(b1, dtype=np.float32))
    b2 = np.ascontiguousarray(np.asarray(b2, dtype=np.float32))
    wv = np.ascontiguousarray(np.asarray(Wv, dtype=np.float32).reshape(U))
    in_maps = []
    for b in range(B):
        in_maps.append({
            "enc": np.ascontiguousarray(np.asarray(encoder_output[b], dtype=np.float32)),
            "dec": np.ascontiguousarray(np.asarray(decoder_output[b], dtype=np.float32)),
            "mask": np.ascontiguousarray(np.asarray(mask[b]).astype(np.uint8)),
            "w1": W1, "b1": b1, "w2": W2, "b2": b2, "wv": wv,
        })
    res = run_bass_kernel_spmd(nc, in_maps, core_ids=list(range(N_CORES)),
                               trace=_trace)
    context = np.stack([res.results[b]["ctx_out"] for b in range(B)])
    weights = np.stack([res.results[b]["w_out"] for b in range(B)])[..., None]
    if _trace:
        kernel._last_result = res
    return context.astype(np.float32), weights.astype(np.float32)
